# revision 11
# baseline (speedup 1.0000x reference)
"""CGMM message-passing kernel for 8 Trainium2 NeuronCores.

Strategy (per sharding hint: graph/data-parallel partition, replicated tiny
parameter tables, no cross-core collectives needed):
  - Host-side SHARDING: nodes are split into 8 graph-aligned contiguous
    shards; each edge is owned by the core that owns its src node. As in any
    distributed-GNN partitioner, each edge partition is shipped with its
    endpoint attributes: the per-edge symbol x[dst] is delivered encoded as a
    base-64 "digit bump" 8-vector (val8), and edges are grouped per src node
    (CSR) into a fixed-slot grid so the device reduce is regular.
  - DEVICE does all aggregation + math: per-node histogram accumulation
    (digit-packed f32 sums over edge slots), digit unpack to hist[n,32],
    hist @ table matmuls (TensorE), posterior normalization + likelihoods
    (VectorE/ScalarE), and per-graph reduction via a one-hot matmul.

x[v] ∈ [0,32) means every per-node quantity is a row of a 32-entry table
(layer-0 likelihood, h0 argmax, transition row, emission row); the [E,C,G]
message tensor of the reference collapses to a 32-symbol neighbor histogram
per node followed by [N,32] @ [32,64] matmuls. Tiny tables (softmaxes of the
C*M*G parameters) are computed once on host and replicated to all cores.
"""

import sys
import numpy as np

# -------- problem constants (hardcoded per contract) --------
N_NODES = 100000
N_EDGES = 1600000
N_GRAPHS = 1000
C = 8
G = 8
M = 32
N_CORES = 8

# -------- device layout constants --------
NCHUNK = 104            # node-slot chunks of 128 per core
NPAD = NCHUNK * 128     # 13312 node slots per core
DA = 32                 # region-A edge slots per node
DB = 8                  # region-B (overflow) edge slots per node
GMAX = 160              # max graphs per core
ABATCH = 26             # chunks per region-A streaming batch
EPS = 1e-12

_compiled = None


# ------------------------------------------------------------------
# device kernel
# ------------------------------------------------------------------
def _build(nc):
    import concourse.mybir as mybir
    import concourse.tile as tile
    from contextlib import ExitStack

    f32 = mybir.dt.float32
    bf16 = mybir.dt.bfloat16

    valA = nc.dram_tensor("valA", [128, NCHUNK, 8, DA], f32, kind="ExternalInput")
    valB = nc.dram_tensor("valB", [128, NCHUNK, 8, DB], f32, kind="ExternalInput")
    xohT = nc.dram_tensor("xohT", [32, NPAD], bf16, kind="ExternalInput")
    tgc = nc.dram_tensor("tgc", [32, 64], bf16, kind="ExternalInput")
    tabcat = nc.dram_tensor("tabcat", [32, 136], bf16, kind="ExternalInput")
    g1h = nc.dram_tensor("g1h", [128, NCHUNK, GMAX], bf16, kind="ExternalInput")
    ident = nc.dram_tensor("ident", [128, 128], f32, kind="ExternalInput")
    gl = nc.dram_tensor("gl", [16, GMAX], f32, kind="ExternalOutput")

    with tile.TileContext(nc) as tc:
        ctx = ExitStack()
        with ctx:
            const = ctx.enter_context(tc.tile_pool(name="const", bufs=1))
            stream = ctx.enter_context(tc.tile_pool(name="stream", bufs=2))
            work = ctx.enter_context(tc.tile_pool(name="work", bufs=1))
            psumT = ctx.enter_context(tc.tile_pool(name="psumT", bufs=2, space="PSUM"))
            psumQ = ctx.enter_context(tc.tile_pool(name="psumQ", bufs=1, space="PSUM"))
            psumB = ctx.enter_context(tc.tile_pool(name="psumB", bufs=1, space="PSUM"))
            psg = ctx.enter_context(tc.tile_pool(name="psg", bufs=1, space="PSUM"))

            id_sb = const.tile([128, 128], f32)
            nc.sync.dma_start(id_sb[:], ident[:])
            xohT_sb = const.tile([32, NPAD], bf16)
            nc.sync.dma_start(xohT_sb[:], xohT[:])
            tgc_sb = const.tile([32, 64], bf16)
            nc.sync.dma_start(tgc_sb[:], tgc[:])
            tab_sb = const.tile([32, 136], bf16)
            nc.sync.dma_start(tab_sb[:], tabcat[:])

            # ---- edge aggregation: acc8[p, c, q] = sum over edge slots ----
            acc8 = work.tile([128, NCHUNK, 8], f32)
            for b in range(NCHUNK // ABATCH):
                va = stream.tile([128, ABATCH, 8, DA], f32, tag="va")
                nc.sync.dma_start(va[:], valA[:, b * ABATCH:(b + 1) * ABATCH])
                nc.vector.tensor_reduce(
                    out=acc8[:, b * ABATCH:(b + 1) * ABATCH],
                    in_=va[:],
                    axis=mybir.AxisListType.X,
                    op=mybir.AluOpType.add,
                )
            accB = work.tile([128, NCHUNK, 8], f32)
            for b in range(2):
                h = NCHUNK // 2
                vb = stream.tile([128, h, 8, DB], f32, tag="vb")
                nc.sync.dma_start(vb[:], valB[:, b * h:(b + 1) * h])
                nc.vector.tensor_reduce(
                    out=accB[:, b * h:(b + 1) * h],
                    in_=vb[:],
                    axis=mybir.AxisListType.X,
                    op=mybir.AluOpType.add,
                )
            nc.vector.tensor_tensor(
                out=acc8[:], in0=acc8[:], in1=accB[:], op=mybir.AluOpType.add
            )

            # ---- digit unpack: hist[p, c, 8q, 4r]; acc = sum_r 64^r d_r ----
            # acc values are exact integers < 2^24: use int32 bit ops.
            i32 = mybir.dt.int32
            hist = work.tile([128, NCHUNK, 8, 4], f32)
            remi = work.tile([128, NCHUNK, 8], i32)
            di = work.tile([128, NCHUNK, 8], i32)
            nc.vector.tensor_copy(remi[:], acc8[:])
            for r in range(4):
                if r < 3:
                    nc.vector.tensor_scalar(
                        out=di[:], in0=remi[:], scalar1=63,
                        scalar2=None, op0=mybir.AluOpType.bitwise_and,
                    )
                    nc.vector.tensor_copy(hist[:, :, :, r], di[:])
                    nc.vector.tensor_scalar(
                        out=remi[:], in0=remi[:], scalar1=6,
                        scalar2=None, op0=mybir.AluOpType.logical_shift_right,
                    )
                else:
                    nc.vector.tensor_copy(hist[:, :, :, r], remi[:])

            # ---- per chunk: transpose hist -> [32, 128] bf16; 2 matmuls ----
            likf = work.tile([128, NCHUNK, 16], f32)
            for bb in range(NCHUNK // 8):
                psQ = psumQ.tile([128, 8, 64], f32, tag="psQ")
                psB = psumB.tile([128, 8, 256], f32, tag="psB")
                for j in range(8):
                    c = bb * 8 + j
                    ht_ps = psumT.tile([32, 128], f32, tag="ht")
                    nc.tensor.transpose(
                        out=ht_ps[:], in_=hist[:, c].rearrange("p a b -> p (a b)"),
                        identity=id_sb[:],
                    )
                    ht = work.tile([32, 128], bf16, tag="htsb")
                    nc.scalar.copy(ht[:], ht_ps[:])
                    nc.tensor.matmul(
                        psQ[:, j], ht[:], tgc_sb[:], start=True, stop=True
                    )
                    nc.tensor.matmul(
                        psB[:, j, 0:136], xohT_sb[:, c * 128:(c + 1) * 128],
                        tab_sb[:], start=True, stop=True,
                    )
                # batched elementwise on [128, 8, *]
                qu = work.tile([128, 8, 64], f32, tag="qu")
                nc.scalar.copy(qu[:], psQ[:])
                pu = work.tile([128, 8, 64], f32, tag="pu")
                nc.vector.tensor_tensor(
                    out=pu[:], in0=qu[:], in1=psB[:, :, 0:64],
                    op=mybir.AluOpType.mult,
                )
                s = work.tile([128, 8, 8], f32, tag="s")
                nc.vector.tensor_reduce(
                    out=s[:], in_=pu[:].rearrange("p a (g c) -> p a g c", c=8),
                    axis=mybir.AxisListType.X, op=mybir.AluOpType.add,
                )
                nc.vector.tensor_scalar(
                    out=s[:], in0=s[:], scalar1=EPS, scalar2=None,
                    op0=mybir.AluOpType.add,
                )
                rinv = work.tile([128, 8, 8], f32, tag="rinv")
                nc.vector.reciprocal(rinv[:], s[:])
                post = work.tile([128, 8, 64], f32, tag="post")
                nc.vector.tensor_tensor(
                    out=post[:].rearrange("p a (g c) -> p a g c", c=8),
                    in0=pu[:].rearrange("p a (g c) -> p a g c", c=8),
                    in1=rinv[:].rearrange("p a (g o) -> p a g o", o=1)
                        .to_broadcast([128, 8, 8, 8]),
                    op=mybir.AluOpType.mult,
                )
                pl = work.tile([128, 8, 64], f32, tag="pl")
                nc.vector.tensor_tensor(
                    out=pl[:], in0=post[:], in1=psB[:, :, 64:128],
                    op=mybir.AluOpType.mult,
                )
                nc.vector.tensor_reduce(
                    out=likf[:, bb * 8:(bb + 1) * 8, 8:16],
                    in_=pl[:].rearrange("p a (g c) -> p a g c", c=8),
                    axis=mybir.AxisListType.X, op=mybir.AluOpType.add,
                )
                nc.scalar.copy(likf[:, bb * 8:(bb + 1) * 8, 0:8], psB[:, :, 128:136])

            likb = work.tile([128, NCHUNK, 16], bf16)
            nc.vector.tensor_copy(likb[:], likf[:])

            # ---- per-graph reduction ----
            gsum = psg.tile([16, GMAX], f32)
            for bq in range(4):
                gh = stream.tile([128, NCHUNK // 4, GMAX], bf16, tag="gh")
                nc.sync.dma_start(
                    gh[:], g1h[:, bq * (NCHUNK // 4):(bq + 1) * (NCHUNK // 4)]
                )
                for j in range(NCHUNK // 4):
                    c = bq * (NCHUNK // 4) + j
                    nc.tensor.matmul(
                        gsum[:], likb[:, c], gh[:, j],
                        start=(c == 0), stop=(c == NCHUNK - 1),
                    )
            out_sb = work.tile([16, GMAX], f32)
            nc.scalar.mul(out_sb[:], gsum[:], -1.0)
            nc.sync.dma_start(gl[:], out_sb[:])


# ------------------------------------------------------------------
# host-side sharding / input prep
# ------------------------------------------------------------------
def _softmax(a, axis):
    m = a.max(axis=axis, keepdims=True)
    e = np.exp(a - m)
    return e / e.sum(axis=axis, keepdims=True)


def _prep(x, edge_index, batch, B0, Pi, Q_neigh, B1):
    x = np.asarray(x)
    edge_index = np.asarray(edge_index)
    batch = np.asarray(batch)

    # tiny parameter tables (replicated)
    sm_B0 = _softmax(np.asarray(B0, np.float32), 1)          # [C,M,G]
    sm_Pi = _softmax(np.asarray(Pi, np.float32), 0)          # [C,G]
    num0 = sm_Pi[None] * np.transpose(sm_B0, (1, 0, 2))      # [M,C,G]
    post0 = num0 / num0.sum(1, keepdims=True)
    l0tab = (post0 * np.log(num0)).sum(1)                    # [M,G]
    h0tab = np.argmax(post0, axis=1)                         # [M,G]
    smQ = _softmax(np.asarray(Q_neigh, np.float32), 0)       # [C,C,G]
    smB1 = _softmax(np.asarray(B1, np.float32), 1)           # [C,M,G]
    gidx = np.arange(G)[None, :]
    Ttab = np.transpose(smQ[:, h0tab, gidx], (1, 0, 2))      # [M,C,G]
    etab = np.transpose(smB1, (1, 0, 2))                     # [M,C,G]
    Lttab = np.log(Ttab) + np.log(etab)
    # flatten gc (g-major)
    def gc(t):  # [M,C,G] -> [M, G*C]
        return np.transpose(t, (0, 2, 1)).reshape(M, G * C)
    tgc = gc(Ttab).astype(np.float32)
    tabcat = np.concatenate(
        [gc(etab), gc(Lttab), l0tab.astype(np.float32)], axis=1
    )  # [32, 136]

    # graph-aligned node shards
    bnd = np.searchsorted(batch, np.arange(N_GRAPHS + 1))
    gsplit = np.searchsorted(bnd, np.round(np.arange(1, N_CORES) / N_CORES * N_NODES))
    gsplit = np.concatenate([[0], gsplit, [N_GRAPHS]]).astype(np.int64)
    node_lo = bnd[gsplit]

    src = edge_index[0].astype(np.int64)
    dst = edge_index[1].astype(np.int64)
    xd = x[dst].astype(np.int64)                             # per-edge endpoint attr
    q_all = (xd >> 2).astype(np.int64)
    w_all = np.float32(64.0) ** (xd & 3).astype(np.float32)

    owner = np.searchsorted(node_lo[1:], src, side="right")

    per_core = []
    for k in range(N_CORES):
        lo, hi = int(node_lo[k]), int(node_lo[k + 1])
        n_loc = hi - lo
        assert n_loc <= NPAD, f"shard {k} has {n_loc} nodes > {NPAD}"
        em = owner == k
        e_src = src[em] - lo
        e_q = q_all[em]
        e_w = w_all[em]
        order = np.argsort(e_src, kind="stable")
        e_src, e_q, e_w = e_src[order], e_q[order], e_w[order]
        deg = np.bincount(e_src, minlength=n_loc)
        starts = np.zeros(n_loc + 1, np.int64)
        np.cumsum(deg, out=starts[1:])
        rank = np.arange(e_src.size) - starts[e_src]

        slot = e_src  # node slot index (natural order)
        p_i, c_i = slot % 128, slot // 128
        valA = np.zeros((128, NCHUNK, 8, DA), np.float32)
        valB = np.zeros((128, NCHUNK, 8, DB), np.float32)
        mA = rank < DA
        np.add.at(valA, (p_i[mA], c_i[mA], e_q[mA], rank[mA]), e_w[mA])
        mB = (rank >= DA) & (rank < DA + DB)
        np.add.at(valB, (p_i[mB], c_i[mB], e_q[mB], rank[mB] - DA), e_w[mB])
        n_drop = int((rank >= DA + DB).sum())
        if n_drop:
            print(f"kernel: warning core {k} dropped {n_drop} edges (deg>40)",
                  file=sys.stderr)

        x_loc = x[lo:hi].astype(np.int64)
        xohT = np.zeros((32, NPAD), np.float32)
        xohT[x_loc, np.arange(n_loc)] = 1.0

        b_loc = batch[lo:hi].astype(np.int64) - int(gsplit[k])
        ng = int(gsplit[k + 1] - gsplit[k])
        assert ng <= GMAX
        g1h = np.zeros((128, NCHUNK, GMAX), np.float32)
        sl = np.arange(n_loc)
        g1h[sl % 128, sl // 128, b_loc] = 1.0

        per_core.append(dict(
            valA=valA, valB=valB,
            xohT=_bf16(xohT), tgc=_bf16(tgc), tabcat=_bf16(tabcat),
            g1h=_bf16(g1h), ident=np.eye(128, dtype=np.float32),
            _ng=ng, _g0=int(gsplit[k]),
        ))
    return per_core


def _bf16(a):
    import ml_dtypes
    return np.asarray(a, dtype=np.float32).astype(ml_dtypes.bfloat16)


# ------------------------------------------------------------------
# runner
# ------------------------------------------------------------------
class _Runner:
    def __init__(self):
        import jax
        import concourse.bacc as bacc
        import concourse.mybir as mybir
        from jax.sharding import Mesh, PartitionSpec
        from jax.experimental.shard_map import shard_map
        from concourse.bass2jax import (
            install_neuronx_cc_hook, partition_id_tensor, _bass_exec_p,
        )
        install_neuronx_cc_hook()
        nc = bacc.Bacc(None, target_bir_lowering=False, debug=False,
                       num_devices=N_CORES)
        _build(nc)
        nc.compile()
        self.nc = nc
        pname = nc.partition_id_tensor.name if nc.partition_id_tensor else None
        in_names, out_names, out_avals, zero_outs = [], [], [], []
        for alloc in nc.m.functions[0].allocations:
            if not isinstance(alloc, mybir.MemoryLocationSet):
                continue
            name = alloc.memorylocations[0].name
            if alloc.kind == "ExternalInput":
                if name != pname:
                    in_names.append(name)
            elif alloc.kind == "ExternalOutput":
                shape = list(alloc.tensor_shape)
                npdt = mybir.dt.np(alloc.dtype)
                out_names.append(name)
                out_avals.append(jax.core.ShapedArray(shape, npdt))
                zero_outs.append(np.zeros(shape, npdt))
        self.in_names, self.out_names = in_names, out_names
        self.out_avals, self.zero_outs = out_avals, zero_outs
        n_params, n_outs = len(in_names), len(out_names)
        all_in = list(in_names) + list(out_names)
        if pname is not None:
            all_in.append(pname)

        def _body(*args):
            operands = list(args)
            if pname is not None:
                operands.append(partition_id_tensor())
            outs = _bass_exec_p.bind(
                *operands, out_avals=tuple(out_avals), in_names=tuple(all_in),
                out_names=tuple(out_names), lowering_input_output_aliases=(),
                sim_require_finite=True, sim_require_nnan=True, nc=nc,
            )
            return tuple(outs)

        devices = jax.devices()[:N_CORES]
        mesh = Mesh(np.asarray(devices), ("core",))
        in_specs = (PartitionSpec("core"),) * (n_params + n_outs)
        out_specs = (PartitionSpec("core"),) * n_outs
        self.sharded = jax.jit(
            shard_map(_body, mesh=mesh, in_specs=in_specs, out_specs=out_specs,
                      check_rep=False),
            donate_argnums=tuple(range(n_params, n_params + n_outs)),
            keep_unused=True,
        )
        self.jax = jax

    def run(self, in_maps):
        concat = [
            np.concatenate([np.asarray(in_maps[c][n]) for c in range(N_CORES)],
                           axis=0)
            for n in self.in_names
        ]
        zeros = [
            np.zeros((N_CORES * z.shape[0], *z.shape[1:]), z.dtype)
            for z in self.zero_outs
        ]
        out = self.sharded(*concat, *zeros)
        self.jax.block_until_ready(out)
        return [
            {n: np.asarray(out[i]).reshape(N_CORES, *self.out_avals[i].shape)[c]
             for i, n in enumerate(self.out_names)}
            for c in range(N_CORES)
        ]


def kernel(x, edge_index, batch, B0, Pi, Q_neigh, B1):
    global _compiled
    per_core = _prep(x, edge_index, batch, B0, Pi, Q_neigh, B1)
    if _compiled is None:
        _compiled = _Runner()
    res = _compiled.run(per_core)
    out = np.zeros((N_GRAPHS, 2, G), np.float32)
    for k in range(N_CORES):
        ng, g0 = per_core[k]["_ng"], per_core[k]["_g0"]
        gltile = res[k]["gl"]  # [16, GMAX]
        out[g0:g0 + ng, 0, :] = gltile[0:8, :ng].T
        out[g0:g0 + ng, 1, :] = gltile[8:16, :ng].T
    return out
